# revision 16
# baseline (speedup 1.0000x reference)
"""Trainium2 Bass kernel for nn_GroupedLinear (16-group LayerNorm+Linear).

Problem: x [1024, 8, 64, 64] fp32; per group g (16 groups of 64 channels):
  X_g = contiguous 2M-element chunk g viewed row-major as [32768, 64]
  Y_g = LayerNorm(X_g) * gamma_g + beta_g  @ W_g^T + b_g      [32768, 64]
  out chunk g = Y_g^T  (contiguous [64, 32768] block of the output)

Sharding: expert-parallel, 2 groups per core across 8 cores. Each core's
input (2 x 8MB) and output (16.8MB) are disjoint contiguous DRAM blocks;
no collectives.

Per-core dataflow (512-row macro-tile, 64 iterations):
  DMA in fp32 [128p=4rows, 2g, 4blk, 64ch] (1KB contiguous per partition)
  -> bn_stats (8 segments in one op) -> bn_aggr -> sqrt+reciprocal (rstd)
  -> fused (x-mu)*rstd tensor_scalar per (g,blk), cast to bf16
  -> 4x PE transpose of [128, (2g,64ch)] stripes -> PSUM [128, 512]
     (partitions = stacked g0|g1 channels, free = rows)
  -> ACT copy PSUM->SBUF bf16
  -> one matmul with block-diag [128,128] bf16 weights (both groups at once,
     gamma folded into W), N=512 -> PSUM
  -> tensor_scalar add bias (W@beta+b folded) + un-permute rows -> SBUF fp32
  -> DMA out (2KB contiguous per partition)
"""

import sys

for _p in ("/opt/trn_rl_repo", "/opt/pypackages"):
    if _p not in sys.path:
        sys.path.insert(0, _p)

import numpy as np
import ml_dtypes

G_TOTAL = 16
N_CORES = 8
G_PER_CORE = G_TOTAL // N_CORES  # 2
IN_G = 64
OUT_G = 64
ROWS = 8 * 64 * 64  # 32768 rows per group
MACRO = 1024  # rows per macro-tile
NB = MACRO // 128  # 8 row-blocks per macro (partition p = rows NB*p+bb)
NMAC = ROWS // MACRO  # 32
EPS = 1e-6

_CACHE = {}


def _build_bass(rep=1):
    import concourse.bacc as bacc
    import concourse.bass as bass
    import concourse.tile as tile
    from concourse import mybir

    nc = bacc.Bacc(None, target_bir_lowering=False)

    x = nc.dram_tensor("x", [G_PER_CORE, ROWS, IN_G], mybir.dt.float32,
                       kind="ExternalInput")
    wb = nc.dram_tensor("wb", [128, 128], mybir.dt.bfloat16,
                        kind="ExternalInput")
    tb = nc.dram_tensor("tb", [128, 1], mybir.dt.float32,
                        kind="ExternalInput")
    ident = nc.dram_tensor("ident", [128, 128], mybir.dt.bfloat16,
                           kind="ExternalInput")
    out = nc.dram_tensor("out", [128, ROWS], mybir.dt.bfloat16,
                         kind="ExternalOutput")

    F = mybir.ActivationFunctionType
    A = mybir.AluOpType

    with tile.TileContext(nc) as tc:
        with (
            tc.tile_pool(name="singles", bufs=1) as singles,
            tc.tile_pool(name="xload", bufs=4) as xload,
            tc.tile_pool(name="statp", bufs=4) as statp,
            tc.tile_pool(name="mvp", bufs=4) as mvp,
            tc.tile_pool(name="rstdp", bufs=4) as rstdp,
            tc.tile_pool(name="xnp", bufs=4) as xnp,
            tc.tile_pool(name="xtsp", bufs=4) as xtsp,
            tc.tile_pool(name="youtp", bufs=4) as youtp,
            tc.tile_pool(name="xtpp", bufs=2, space="PSUM") as xtpp,
            tc.tile_pool(name="ypp", bufs=3, space="PSUM") as ypp,
        ):
            sb_wb = singles.tile([128, 128], mybir.dt.bfloat16)
            sb_tb = singles.tile([128, 1], mybir.dt.float32)
            sb_id = singles.tile([128, 128], mybir.dt.bfloat16)
            sb_eps = singles.tile([128, 1], mybir.dt.float32)
            nc.sync.dma_start(out=sb_wb, in_=wb[:, :])
            nc.sync.dma_start(out=sb_tb, in_=tb[:, :])
            nc.sync.dma_start(out=sb_id, in_=ident[:, :])
            nc.vector.memset(sb_eps, EPS)

            for m in range(NMAC * rep):
                m = m % NMAC
                r0 = m * MACRO
                # ---- load (one DMA, both groups): partition p holds rows
                # NB*p .. NB*p+NB-1 of each group (contiguous per partition)
                x_t = xload.tile([128, G_PER_CORE, NB, IN_G],
                                 mybir.dt.float32)
                # 2KB contiguous per (partition, group): row r = 8*p + b
                nc.sync.dma_start(
                    out=x_t,
                    in_=x[:, r0:r0 + MACRO, :].rearrange(
                        "g (p b) c -> p g b c", p=128),
                )

                # ---- stats: ONE segmented bn_stats (8 segments = blocks,
                # (c,g) interleaved within each segment -> even stream
                # positions = g0, odd = g1, 6 outputs per segment)
                st = statp.tile([128, NB, 6], mybir.dt.float32)
                for bb in range(NB):
                    in3 = x_t[:, :, bb, :].rearrange("p g c -> p c g")
                    nc.vector.add_instruction(
                        mybir.InstBNStats(
                            name=nc.get_next_instruction_name(),
                            ins=[nc.vector.lower_ap(in3)],
                            outs=[nc.vector.lower_ap(st[:, bb, :])],
                        )
                    )
                # rstd = 1/sqrt((count*var)/64 + eps); layout [128, bb, g]
                rstd = rstdp.tile([128, NB, 2], mybir.dt.float32)
                nc.scalar.activation(out=rstd, in_=st[:, :, 2:6:3],
                                     func=F.Abs_reciprocal_sqrt,
                                     bias=sb_eps[:, 0:1],
                                     scale=1.0 / IN_G)

                # ---- normalize (x - mu) * rstd, cast to bf16, as two big
                # broadcast tensor_tensor ops (stride-0 inner dim broadcasts
                # the per-(blk,g) scalar across the 64 channels).
                # layout [128, blk, g, c] so each transpose stripe is a
                # contiguous 128-wide free range.
                xn = xnp.tile([128, NB, G_PER_CORE, IN_G], mybir.dt.bfloat16)
                xn_v = xn.rearrange("p b g c -> p g b c")
                x_v = x_t[:, :, :, :]  # [128, g, b, c]
                st_ap = st[:, :, :]
                rstd_ap = rstd[:, :, :]
                mu_b = bass.AP(
                    tensor=st_ap.tensor, offset=st_ap.offset + 1,
                    ap=[st_ap.ap[0], [3, G_PER_CORE], [6, NB], [0, IN_G]],
                )
                rstd_b = bass.AP(
                    tensor=rstd_ap.tensor, offset=rstd_ap.offset,
                    ap=[rstd_ap.ap[0], [1, G_PER_CORE], [2, NB], [0, IN_G]],
                )
                nc.vector.tensor_sub(xn_v, x_v, mu_b)
                nc.gpsimd.tensor_mul(xn_v, xn_v, rstd_b)

                # ---- transpose NB stripes -> PSUM [128, MACRO]
                # stripe bb input [128, (2g, 64ch)] -> psum partitions
                # g*64+c, free = 128 rows (row = NB*q+bb)
                xtp = xtpp.tile([128, MACRO], mybir.dt.bfloat16)
                for bb in range(NB):
                    nc.tensor.transpose(
                        out=xtp[:, bb * 128:(bb + 1) * 128],
                        in_=xn[:, bb, :, :].rearrange("p g c -> p (g c)"),
                        identity=sb_id,
                    )

                # ---- PSUM -> SBUF bf16 (ACT), un-permuting rows via a
                # strided read: xtp col (b,q) -> xts col r=8q+b
                xts = xtsp.tile([128, MACRO], mybir.dt.bfloat16)
                xtp_v = xtp.rearrange("p (b q) -> p q b", b=NB)
                nc.scalar.activation(out=xts.rearrange("p (q b) -> p q b",
                                                       b=NB),
                                     in_=xtp_v, func=F.Copy)

                # ---- matmul: both groups per op via block-diag weights,
                # N=512 per PSUM bank. rhs is read through a (q b)-permuted
                # AP so PSUM columns come out in natural row order r=8q+b
                # (free un-permute on the PE).
                yp = ypp.tile([128, MACRO], mybir.dt.float32)
                for k in range(MACRO // 512):
                    nc.tensor.matmul(yp[:, k * 512:(k + 1) * 512],
                                     lhsT=sb_wb,
                                     rhs=xts[:, k * 512:(k + 1) * 512],
                                     start=True, stop=True)

                # ---- bias add + cast bf16 (rows already in natural order)
                y_t = youtp.tile([128, MACRO], mybir.dt.bfloat16)
                nc.scalar.activation(out=y_t, in_=yp, func=F.Identity,
                                     bias=sb_tb[:, 0:1], scale=1.0)

                nc.gpsimd.dma_start(out=out[:, r0:r0 + MACRO], in_=y_t)

    nc.finalize()
    return nc


def _get_nc(rep=1):
    key = ("nc", rep)
    if key not in _CACHE:
        _CACHE[key] = _build_bass(rep)
    return _CACHE[key]


def _make_in_maps(x, ln_gamma, ln_beta, W, b):
    bf16 = ml_dtypes.bfloat16
    xg = np.ascontiguousarray(x.reshape(G_TOTAL, ROWS, IN_G))
    in_maps = []
    for c in range(N_CORES):
        gs = [G_PER_CORE * c + g for g in range(G_PER_CORE)]
        wbc = np.zeros((128, 128), np.float32)
        tbc = np.zeros((128, 1), np.float32)
        for g_local, g in enumerate(gs):
            Wp = W[g] * ln_gamma[g][None, :]  # [out, in] gamma folded
            lo = g_local * 64
            wbc[lo:lo + 64, lo:lo + 64] = Wp.T  # lhsT[k=in, m=out]
            tbc[lo:lo + 64, 0] = W[g] @ ln_beta[g] + b[g]
        in_maps.append({
            "x": np.ascontiguousarray(xg[gs[0]:gs[-1] + 1]),
            "wb": wbc.astype(bf16),
            "tb": tbc,
            "ident": np.eye(128, dtype=np.float32).astype(bf16),
        })
    return in_maps


def _run(in_maps, trace=False):
    from concourse.bass_utils import run_bass_kernel_spmd
    nc = _get_nc()
    return run_bass_kernel_spmd(nc, in_maps, list(range(N_CORES)),
                                trace=trace)


def bench(in_maps, rep, iters=12):
    """Time repeated on-device executions of the rep-times-unrolled kernel.

    Returns list of per-call wall times (s). Per-iteration kernel time is
    estimated by the caller from the difference between two rep values.
    """
    import time
    import jax
    import jax.numpy as jnp
    import numpy as np_
    from jax.sharding import Mesh, PartitionSpec
    from jax.experimental.shard_map import shard_map
    from concourse import bass2jax
    from concourse import mybir

    bass2jax.install_neuronx_cc_hook()
    nc = _get_nc(rep)

    partition_name = (nc.partition_id_tensor.name
                      if nc.partition_id_tensor else None)
    in_names, out_names, out_avals = [], [], []
    zero_shapes = []
    for alloc in nc.m.functions[0].allocations:
        if not isinstance(alloc, mybir.MemoryLocationSet):
            continue
        name = alloc.memorylocations[0].name
        if alloc.kind == "ExternalInput":
            if name != partition_name:
                in_names.append(name)
        elif alloc.kind == "ExternalOutput":
            out_names.append(name)
            shape = tuple(alloc.tensor_shape)
            dtype = mybir.dt.np(alloc.dtype)
            out_avals.append(jax.core.ShapedArray(shape, dtype))
            zero_shapes.append((shape, dtype))
    n_params = len(in_names)
    all_names = list(in_names) + out_names
    if partition_name is not None:
        all_names.append(partition_name)

    def _body(*args):
        operands = list(args)
        if partition_name is not None:
            operands.append(bass2jax.partition_id_tensor())
        outs = bass2jax._bass_exec_p.bind(
            *operands,
            out_avals=tuple(out_avals),
            in_names=tuple(all_names),
            out_names=tuple(out_names),
            lowering_input_output_aliases=(),
            sim_require_finite=True,
            sim_require_nnan=True,
            nc=nc,
        )
        return tuple(outs)

    n_cores = len(in_maps)
    devices = jax.devices()[:n_cores]
    mesh = Mesh(np_.asarray(devices), ("core",))
    nzero = len(zero_shapes)
    in_specs = (PartitionSpec("core"),) * (n_params + nzero)
    out_specs = (PartitionSpec("core"),) * len(out_names)
    donate = tuple(range(n_params, n_params + nzero))
    sharded = jax.jit(
        shard_map(_body, mesh=mesh, in_specs=in_specs,
                  out_specs=out_specs, check_rep=False),
        donate_argnums=donate, keep_unused=True)

    concat_in = [
        jax.device_put(
            np_.concatenate([np_.asarray(in_maps[c][name])
                             for c in range(n_cores)], axis=0))
        for name in in_names
    ]

    def make_zeros():
        return [
            jnp.zeros((shape[0] * n_cores,) + tuple(shape[1:]), dtype)
            for shape, dtype in zero_shapes
        ]

    times = []
    for i in range(iters):
        zs = [jax.device_put(z) for z in make_zeros()]
        for z in zs:
            z.block_until_ready()
        t0 = time.perf_counter()
        outs = sharded(*concat_in, *zs)
        for o in outs:
            o.block_until_ready()
        times.append(time.perf_counter() - t0)
    return times


def kernel(x, ln_gamma, ln_beta, W, b):
    x = np.asarray(x, np.float32)
    ln_gamma = np.asarray(ln_gamma, np.float32)
    ln_beta = np.asarray(ln_beta, np.float32)
    W = np.asarray(W, np.float32)
    b = np.asarray(b, np.float32)

    in_maps = _make_in_maps(x, ln_gamma, ln_beta, W, b)
    res = _run(in_maps, trace=False)
    outs = [np.asarray(r["out"]).astype(np.float32) for r in res.results]
    full = np.concatenate(outs, axis=0)  # [1024, 32768]
    return full.reshape(1024, 8, 64, 64)

